# revision 44
# baseline (speedup 1.0000x reference)
"""Trainium2 Bass kernel for a post-LN transformer encoder block (full IO).

Sharding: 8-way data parallel over (batch, sequence-half): core c handles
batch c//2 and query rows [(c%2)*1024, (c%2)*1024+1024). Each core computes
K/V for its whole batch (duplicated across the 2 cores sharing a batch), so
there are no cross-core collectives. For odd cores the host rotates the key
axis by 1024 so every core's queries sit at columns [0, 1024) — one NEFF
serves all 8 cores.

On-chip layout is feature-major ([d, seq]); the host transposes x on the way
in and the output back.

The 1024 queries are processed as two 512-column chunks software-pipelined
against each other: attention for chunk 1 (bound by the scalar engine's exp
throughput) overlaps the wo/LN1/FFN/LN2 stream of chunk 0 (bound by the
TensorEngine), keeping both engines and the HAM clock-gate busy. Within
attention, each head pair's scores matmuls contract over disjoint 64-row
groups of the PE array (concurrent via base-partition-derived tile_position)
and the ctx matmuls for step k are emitted after the scores of step k+1 so
the PE FIFO never head-blocks on exp. Softmax denominators fall out of the
ctx matmul via a ones column interleaved into V. LN istd is computed as
exp(-0.5*ln(var)) so the whole kernel uses the single
natural_log_exp_and_others activation table set (no sqrt table switches);
relu runs on the vector engine, squares and LN applies on gpsimd.
"""

import sys
import numpy as np

for _p in ("/root/.axon_site", "/root/.axon_site/_ro/trn_rl_repo",
           "/root/.axon_site/_ro/pypackages", "/opt/trn_rl_repo"):
    if _p not in sys.path:
        sys.path.append(_p)

import ml_dtypes

B, S, D, H, DFF = 4, 2048, 1024, 16, 4096
DK = D // H            # 64
EPS = 1e-9
N_CORES = 8
M = S // 2             # queries per core
NB = ml_dtypes.bfloat16

KI = D // 128          # 8 contraction tiles over the model dim
KT = S // 128          # 16 key tiles
QC = M // 512          # 2 query chunks
NP = H // 2            # 8 head pairs
VST = DK + 1           # 65: V head block incl. ones column


def build(mask_has_zeros: bool):
    import concourse.bass as bass
    import concourse.mybir as mybir
    import concourse.tile as tile
    from concourse import bacc
    import contextlib

    BF = mybir.dt.bfloat16
    F32 = mybir.dt.float32
    PF32 = mybir.dt.float32
    ACTF = mybir.ActivationFunctionType
    AL = mybir.AluOpType

    nc = bacc.Bacc("TRN2", target_bir_lowering=False, debug=False,
                   num_devices=N_CORES)

    xtb_d = nc.dram_tensor("xtb", [D, S], BF, kind="ExternalInput").ap()
    wq_d = nc.dram_tensor("wqb", [D, D], BF, kind="ExternalInput").ap()
    wk_d = nc.dram_tensor("wkb", [D, D], BF, kind="ExternalInput").ap()
    wv_d = nc.dram_tensor("wvb", [D, D], BF, kind="ExternalInput").ap()
    wo_d = nc.dram_tensor("wob", [D, D], BF, kind="ExternalInput").ap()
    w1_d = nc.dram_tensor("w1b", [D, DFF], BF, kind="ExternalInput").ap()
    w2_d = nc.dram_tensor("w2b", [DFF, D], BF, kind="ExternalInput").ap()
    mb_d = nc.dram_tensor("maskb", [128, KT], F32, kind="ExternalInput").ap()
    # cvec per LN i: [g, -g/D, g*EPS+bt]
    cv_d = nc.dram_tensor("cvec", [1, 8], F32, kind="ExternalInput").ap()
    out_d = nc.dram_tensor("outT", [D, M], F32, kind="ExternalOutput").ap()
    scratch_d = nc.dram_tensor("lnrows", [2, M], BF).ap()

    with tile.TileContext(nc) as tc:
        with contextlib.ExitStack() as ctx:
            big = ctx.enter_context(tc.tile_pool(name="big", bufs=1))
            mid = ctx.enter_context(tc.tile_pool(name="mid", bufs=1))
            ps = ctx.enter_context(tc.tile_pool(name="ps", bufs=1, space="PSUM"))

            # m2 ring (2KB slots): xtb2 halves, then attention cxr staging
            def m2(shape, dt, name):
                return big.tile(shape, dt, tag="m2", bufs=16, name=name)

            def ktt(name):
                return big.tile([128, 2048], BF, tag="kt", bufs=KI, name=name)

            def qtt(name):
                return big.tile([128, 1024], BF, tag="qt", bufs=KI, name=name)

            def vtile(name):
                return big.tile([128, H * VST], BF, tag="vt", bufs=KT, name=name)

            def trunkt(name):
                return big.tile([128, 512], F32, tag="trunk", bufs=8, name=name)

            def ctxtt(name):
                return mid.tile([128, 512], BF, tag="ctxt", bufs=9, name=name)

            def x2bt(name):
                return mid.tile([128, 512], BF, tag="x2b", bufs=9, name=name)

            def ffbt(name):
                return mid.tile([128, 512], BF, tag="ffb", bufs=9, name=name)

            def wst(name):
                return mid.tile([128, 1024], BF, tag="wst", bufs=10, name=name)

            def ett(name):
                return mid.tile([128, 512], BF, tag="et", bufs=5, name=name)

            def xbt(name):
                return mid.tile([128, 512], BF, tag="xb", bufs=3, name=name)

            def sqt(name):
                return mid.tile([128, 512], BF, tag="sq", bufs=3, name=name)

            def rxbt(name):
                return mid.tile([128, 512], BF, tag="rxb", bufs=4, name=name)

            def drow(name):
                return mid.tile([1, 512], F32, tag="drow", bufs=2, name=name)

            def dbfr(name):
                return mid.tile([1, 512], BF, tag="dbf", bufs=2, name=name)

            def rowt(name):
                return mid.tile([1, 512], F32, tag="rows", bufs=2, name=name)

            def rowbt(name):
                return mid.tile([1, 512], BF, tag="rowsb", bufs=2, name=name)

            def abt(name):
                return mid.tile([128, 2, 512], BF, tag="ab", bufs=2, name=name)

            def psA(name):
                return ps.tile([128, 512], PF32, tag="psA", bufs=2, name=name)

            def psS(name):
                return ps.tile([128, 512], PF32, tag="psS", bufs=3, name=name)

            def psC(name):
                return ps.tile([128, 512], PF32, tag="psC", bufs=2, name=name)

            def psT(name):
                # single dedicated stats bank: s1 at partition 0, s2 at 64
                return ps.tile([128, 512], PF32, tag="psT", bufs=1, name=name)

            # ---------- constants ----------
            ones_row = mid.tile([1, 64], BF, tag="ones_r", bufs=1)
            nc.vector.memset(ones_row, 1.0)
            ones64 = mid.tile([128, 64], BF, tag="ones64", bufs=1)
            nc.vector.memset(ones64, 1.0)
            ones_col = mid.tile([128, 1], BF, tag="ones_c", bufs=1)
            nc.vector.memset(ones_col, 1.0)
            cvec = mid.tile([1, 8], F32, tag="cvec", bufs=1)
            nc.sync.dma_start(out=cvec, in_=cv_d)
            if mask_has_zeros:
                mbt = mid.tile([128, KT], F32, tag="mbt", bufs=1)
                nc.sync.dma_start(out=mbt, in_=mb_d)

            # ---------- load xT as 16 half-seq tiles ----------
            xtb2 = []
            for t in range(2 * KI):
                ki, hh = t // 2, t % 2
                xt = m2([128, 1024], BF, f"xtb{t}")
                nc.sync.dma_start(
                    out=xt, in_=xtb_d[ki * 128:(ki + 1) * 128,
                                      hh * 1024:(hh + 1) * 1024])
                xtb2.append(xt)

            def run_gen(g):
                for _ in g:
                    pass

            def interleave(gen_a, gen_b, ratio):
                """One gen_a step, then ~ratio gen_b quanta, until both end."""
                sent = object()
                acc = 0.0
                b_done = False
                for _ in gen_a:
                    acc += ratio
                    while not b_done and acc >= 1.0:
                        acc -= 1.0
                        if next(gen_b, sent) is sent:
                            b_done = True
                if not b_done:
                    run_gen(gen_b)

            def stream_w(w_dram, name, col0=0, ncols=D):
                wts = []
                for ki in range(KI):
                    wt = wst(f"{name}{ki}")
                    nc.sync.dma_start(
                        out=wt, in_=w_dram[ki * 128:(ki + 1) * 128,
                                           col0:col0 + ncols])
                    wts.append(wt)
                return wts

            # ---------- projections ----------
            def proj_mo(wts, n_cols, out_tiles, mo, name):
                for cc in range(n_cols // 512):
                    acc = psA(f"{name}ps{mo}_{cc}")
                    for ki in range(KI):
                        nc.tensor.matmul(
                            acc[:, :],
                            wts[ki][:, mo * 128:(mo + 1) * 128],
                            xtb2[ki * 2 + cc // 2][:, (cc % 2) * 512:
                                                   (cc % 2) * 512 + 512],
                            start=(ki == 0), stop=(ki == KI - 1),
                        )
                        if ki == 3:
                            yield
                    nc.vector.tensor_copy(
                        out_tiles[mo][:, cc * 512:(cc + 1) * 512],
                        acc[:, :])

            qt = [qtt(f"qt{i}") for i in range(KI)]
            wqts = stream_w(wq_d, "wq")
            for mo in range(KI):
                run_gen(proj_mo(wqts, M, qt, mo, "q"))

            # ---------- V (sequence-major, head-interleaved + ones col) -----
            wvts = stream_w(wv_d, "wv")
            vt = []
            for k in range(KT):
                v = vtile(f"vt{k}")
                vt.append(v)
                hh = k // 8
                for half in range(2):
                    acc = psA(f"vps{k}_{half}")
                    for ki in range(KI):
                        nc.tensor.matmul(
                            acc[:, :],
                            xtb2[ki * 2 + hh][:, (k % 8) * 128:
                                              (k % 8) * 128 + 128],
                            wvts[ki][:, half * 512:(half + 1) * 512],
                            start=(ki == 0), stop=(ki == KI - 1),
                        )
                    dst = v[:, half * 8 * VST:(half * 8 + 8) * VST].rearrange(
                        "p (h j) -> p h j", j=VST)[:, :, 0:DK]
                    src = acc[:, :].rearrange("p (h j) -> p h j", j=DK)
                    nc.vector.tensor_copy(dst, src)
                ones_view = v[:, :].rearrange(
                    "p (h j) -> p h j", j=VST)[:, :, DK:DK + 1]
                nc.vector.memset(ones_view, 1.0)

            # K projection: tiles 0-1 up front, 2-7 interleaved into attn(c0)
            kt = [ktt(f"kt{i}") for i in range(KI)]
            wkts = stream_w(wk_d, "wk")
            for mo in range(2):
                run_gen(proj_mo(wkts, S, kt, mo, "k"))

            wots = stream_w(wo_d, "wo")

            # ---------- attention (one 512-query chunk), as a generator ----
            # Yields are suppressed near each pair boundary so the partner
            # stream's DVE ops don't queue ahead of the stage copies that
            # free the ctx PSUM banks.
            def attn_chunk(c, ctxt_c):
                cs = slice(c * 512, (c + 1) * 512)
                prevfin = None
                for p in range(NP):
                    cps = [psC(f"cps{c}_{p}_{j}") for j in range(2)]
                    pend = None

                    def emit_ctx(pk, ets):
                        for eo in range(2):
                            h = 2 * p + eo
                            nc.tensor.matmul(
                                cps[eo][0:VST, :],
                                vt[pk][:, h * VST:(h + 1) * VST],
                                ets[eo][:, :],
                                start=(pk == 0), stop=(pk == KT - 1),
                            )

                    for k in range(KT):
                        se = psS(f"se{c}_{p}_{k}")
                        so = psS(f"so{c}_{p}_{k}")
                        nc.tensor.matmul(
                            se[:, :],
                            kt[p][0:64, k * 128:(k + 1) * 128],
                            qt[p][0:64, cs],
                            start=True, stop=True,
                        )
                        nc.tensor.matmul(
                            so[:, :],
                            kt[p][64:128, k * 128:(k + 1) * 128],
                            qt[p][64:128, cs],
                            start=True, stop=True,
                        )
                        if pend is not None:
                            emit_ctx(*pend)
                        ee = ett(f"ee{c}_{p}_{k}")
                        eo_t = ett(f"eo{c}_{p}_{k}")
                        bias = mbt[:, k:k + 1] if mask_has_zeros else 0.0
                        nc.scalar.activation(out=ee[:, :], in_=se[:, :],
                                             func=ACTF.Exp, bias=bias,
                                             scale=0.125)
                        nc.scalar.activation(out=eo_t[:, :], in_=so[:, :],
                                             func=ACTF.Exp, bias=bias,
                                             scale=0.125)
                        pend = (k, (ee, eo_t))
                        if k == 2 and prevfin is not None:
                            # deferred: the reciprocals have had time to run,
                            # so the bps matmuls won't head-block the PE
                            attn_finish(*prevfin)
                            prevfin = None
                        yield
                    emit_ctx(*pend)
                    # stage: free the cps banks promptly
                    cxrs = []
                    for j in range(2):
                        cxr = m2([VST, 512], F32, f"cxr{c}_{p}_{j}")
                        nc.vector.tensor_copy(cxr[:, :], cps[j][0:VST, :])
                        cxrs.append(cxr)
                    prevfin = (p, cxrs, ctxt_c)
                    yield
                attn_finish(*prevfin)

            def attn_finish(p, cxrs, ctxt_c):
                for eo in range(2):
                    cxr = cxrs[eo]
                    dbf = dbfr(f"dbf{p}_{eo}")
                    with nc.allow_low_precision(reason="bf16 softmax denom"):
                        nc.vector.reciprocal(out=dbf[:, :],
                                             in_=cxr[64:65, :])
                    bps = psS(f"bps{p}_{eo}")
                    nc.tensor.matmul(bps[0:64, :], ones_row[:, :],
                                     dbf[:, :], start=True, stop=True)
                    with nc.allow_low_precision(reason="ctx stored bf16"):
                        nc.vector.tensor_mul(
                            ctxt_c[p][eo * 64:(eo + 1) * 64, :],
                            bps[0:64, :], cxr[0:64, :])

            # ---------- wo + residual + LN1 (one chunk) ----------
            def ln_combine(stats, g_idx, nm, c):
                s1 = stats[0:1, :]
                s2 = stats[64:65, :]
                sq1 = drow(f"{nm}sq1")
                nc.scalar.activation(out=sq1[:, :], in_=s1,
                                     func=ACTF.Square)
                var = drow(f"{nm}var")
                nc.vector.scalar_tensor_tensor(
                    out=var[:, :], in0=sq1[:, :],
                    scalar=-1.0 / D, in1=s2,
                    op0=AL.mult, op1=AL.add)
                lnv = drow(f"{nm}lnv")
                nc.scalar.activation(out=lnv[:, :], in_=var[:, :],
                                     func=ACTF.Ln, scale=1.0 / (D - 1))
                istd = drow(f"{nm}istd")
                nc.scalar.activation(out=istd[:, :], in_=lnv[:, :],
                                     func=ACTF.Exp, scale=-0.5)
                r_b = rowt(f"{nm}rb")
                nc.vector.tensor_mul(r_b[:, :], s1, istd[:, :])
                nc.vector.tensor_scalar(
                    out=r_b[:, :], in0=r_b[:, :],
                    scalar1=cvec[:, 3 * g_idx + 1:3 * g_idx + 2],
                    scalar2=cvec[:, 3 * g_idx + 2:3 * g_idx + 3],
                    op0=AL.mult, op1=AL.add)
                r_a = rowt(f"{nm}ra")
                nc.vector.tensor_scalar_mul(
                    out=r_a[:, :], in0=istd[:, :],
                    scalar1=cvec[:, 3 * g_idx:3 * g_idx + 1])
                ra_b = rowbt(f"{nm}rab")
                rb_b = rowbt(f"{nm}rbb")
                with nc.allow_low_precision(reason="ln rows bf16"):
                    nc.vector.tensor_copy(ra_b[:, :], r_a[:, :])
                    nc.vector.tensor_copy(rb_b[:, :], r_b[:, :])
                nc.sync.dma_start(out=scratch_d[0:1, c * 512:(c + 1) * 512],
                                  in_=ra_b[:, :])
                nc.sync.dma_start(out=scratch_d[1:2, c * 512:(c + 1) * 512],
                                  in_=rb_b[:, :])
                ab = abt(f"{nm}ab")
                bsrc = bass.AP(tensor=scratch_d.tensor, offset=c * 512,
                               ap=[[0, 128], [M, 2], [1, 512]])
                nc.sync.dma_start(out=ab[:, :, :], in_=bsrc)
                return ab

            def emit_stats(stats, mo, xb, sq):
                nc.tensor.matmul(stats[0:1, :], ones_col[:, :], xb[:, :],
                                 start=(mo == 0), stop=(mo == KI - 1),
                                 skip_group_check=True,
                                 tile_position=(0, 0))
                nc.tensor.matmul(stats[64:65, :], ones_col[:, :], sq[:, :],
                                 start=(mo == 0), stop=(mo == KI - 1),
                                 skip_group_check=True,
                                 tile_position=(0, 64))

            def wo_ln1(c, wot, ctxt_c, trunk_c, x2b_c):
                stats = psT(f"st1_{c}")
                for mo in range(KI):
                    rx = rxbt(f"rx{c}_{mo}")
                    nc.sync.dma_start(
                        out=rx, in_=xtb_d[mo * 128:(mo + 1) * 128,
                                          c * 512:(c + 1) * 512])
                    tr = trunkt(f"tr{c}_{mo}")
                    trunk_c.append(tr)
                    acc = psA(f"aops{c}_{mo}")
                    for ki in range(KI):
                        nc.tensor.matmul(
                            acc[:, :],
                            wot[ki][:, mo * 128:(mo + 1) * 128],
                            ctxt_c[ki][:, :],
                            start=(ki == 0), stop=(ki == KI - 1),
                        )
                        if ki == 3:
                            yield
                    nc.vector.tensor_add(tr[:, :], acc[:, :], rx[:, :])
                    xb = xbt(f"x1b{c}_{mo}")
                    with nc.allow_low_precision(reason="ln stats bf16"):
                        nc.vector.tensor_copy(xb[:, :], tr[:, :])
                        sq = sqt(f"sq1_{c}_{mo}")
                        nc.vector.tensor_mul(sq[:, :], xb[:, :], xb[:, :])
                    emit_stats(stats, mo, xb, sq)
                    yield
                ab = ln_combine(stats, 0, f"ln1_{c}", c)
                yield
                for mo in range(KI):
                    eng = nc.vector if mo % 2 else nc.gpsimd
                    eng.tensor_mul(trunk_c[mo][:, :], trunk_c[mo][:, :],
                                   ab[:, 0, :])
                    eng.tensor_add(trunk_c[mo][:, :], trunk_c[mo][:, :],
                                   ab[:, 1, :])
                    x2 = x2bt(f"x2b{c}_{mo}")
                    x2b_c.append(x2)
                    with nc.allow_low_precision(reason="ffn input bf16"):
                        eng.tensor_copy(x2[:, :], trunk_c[mo][:, :])
                    if mo % 2:
                        yield

            # ---------- FFN + LN2 stats (one chunk) ----------
            def ffn(c, trunk_c, x2b_c, stats):
                for g in range(4):
                    w1ts = []
                    for ki in range(KI):
                        wt = wst(f"w1t{c}_{g}_{ki}")
                        nc.sync.dma_start(
                            out=wt, in_=w1_d[ki * 128:(ki + 1) * 128,
                                             g * 1024:(g + 1) * 1024])
                        w1ts.append(wt)
                    yield
                    ffb = []
                    for fl in range(8):
                        fb = ffbt(f"ffb{c}_{g}_{fl}")
                        ffb.append(fb)
                        acc = psA(f"f1ps{c}_{g}_{fl}")
                        for ki in range(KI):
                            nc.tensor.matmul(
                                acc[:, :],
                                w1ts[ki][:, fl * 128:(fl + 1) * 128],
                                x2b_c[ki][:, :],
                                start=(ki == 0), stop=(ki == KI - 1),
                            )
                            if ki == 3:
                                yield
                        with nc.allow_low_precision(reason="relu bf16"):
                            nc.vector.tensor_scalar_max(
                                out=fb[:, :], in0=acc[:, :], scalar1=0.0)
                        yield
                    w2ts = []
                    for fl in range(8):
                        wt = wst(f"w2t{c}_{g}_{fl}")
                        nc.sync.dma_start(
                            out=wt,
                            in_=w2_d[(g * 8 + fl) * 128:
                                     (g * 8 + fl + 1) * 128, :])
                        w2ts.append(wt)
                    yield
                    for mo in range(KI):
                        acc = psA(f"f2ps{c}_{g}_{mo}")
                        for fl in range(8):
                            nc.tensor.matmul(
                                acc[:, :],
                                w2ts[fl][:, mo * 128:(mo + 1) * 128],
                                ffb[fl][:, :],
                                start=(fl == 0), stop=(fl == 7),
                            )
                            if fl == 3:
                                yield
                        nc.vector.tensor_add(trunk_c[mo][:, :],
                                             trunk_c[mo][:, :], acc[:, :])
                        if g == 3:
                            xb = xbt(f"x2s{c}_{mo}")
                            with nc.allow_low_precision(reason="ln stats"):
                                nc.vector.tensor_copy(xb[:, :],
                                                      trunk_c[mo][:, :])
                                sq = sqt(f"sq2_{c}_{mo}")
                                nc.vector.tensor_mul(sq[:, :], xb[:, :],
                                                     xb[:, :])
                            emit_stats(stats, mo, xb, sq)
                        yield

            def ln2_out(c, trunk_c, stats):
                ab = ln_combine(stats, 1, f"ln2_{c}", c)
                yield
                for mo in range(KI):
                    eng = nc.vector if mo % 2 else nc.gpsimd
                    eng.tensor_mul(trunk_c[mo][:, :], trunk_c[mo][:, :],
                                   ab[:, 0, :])
                    eng.tensor_add(trunk_c[mo][:, :], trunk_c[mo][:, :],
                                   ab[:, 1, :])
                    nc.sync.dma_start(
                        out=out_d[mo * 128:(mo + 1) * 128,
                                  c * 512:(c + 1) * 512],
                        in_=trunk_c[mo][:, :])
                    if mo % 2:
                        yield

            def tail_chunk(c, wot, ctxt_c, trunk_c, x2b_c):
                yield from wo_ln1(c, wot, ctxt_c, trunk_c, x2b_c)
                stats2 = psT(f"st2_{c}")
                yield from ffn(c, trunk_c, x2b_c, stats2)
                yield from ln2_out(c, trunk_c, stats2)

            # ---------- chunk pipeline ----------
            # attn(c0) is interleaved with the K-projection remainder;
            # attn(c1)'s ACT-bound stream is interleaved at emission with
            # chunk 0's PE-bound wo/FFN/LN tail so the PE always has
            # independent matmuls between dependent attention instructions.
            def k_proj_rest():
                for mo in range(2, KI):
                    yield from proj_mo(wkts, S, kt, mo, "k")

            ctxt0 = [ctxtt(f"ctxt0_{i}") for i in range(KI)]
            interleave(attn_chunk(0, ctxt0), k_proj_rest(), 0.7)

            ctxt1 = [ctxtt(f"ctxt1_{i}") for i in range(KI)]
            trunk0, x2b0 = [], []
            # ~112 attention steps vs ~230 tail quanta
            interleave(attn_chunk(1, ctxt1),
                       tail_chunk(0, wots, ctxt0, trunk0, x2b0), 2.1)

            wots2 = stream_w(wo_d, "wo2")
            trunk1, x2b1 = [], []
            run_gen(tail_chunk(1, wots2, ctxt1, trunk1, x2b1))

    nc.compile()
    return nc


_NC_CACHE = {}


def _get_nc(mask_has_zeros: bool):
    if mask_has_zeros not in _NC_CACHE:
        _NC_CACHE[mask_has_zeros] = build(mask_has_zeros)
    return _NC_CACHE[mask_has_zeros]


def _reference_numpy(x, mask, wq, bq, wk, bk, wv, bv, wo, bo, w1, b1, w2, b2,
                     g1, bt1, g2, bt2):
    import math
    out = np.zeros_like(x)

    def ln(v, g, bt):
        mean = v.mean(-1, keepdims=True)
        std = v.std(-1, keepdims=True, ddof=1)
        return g * ((v - mean) / std + EPS) + bt

    for b in range(B):
        xb = x[b]
        q = (xb @ wq + bq).reshape(S, H, DK).transpose(1, 0, 2)
        k = (xb @ wk + bk).reshape(S, H, DK).transpose(1, 0, 2)
        v = (xb @ wv + bv).reshape(S, H, DK).transpose(1, 0, 2)
        ctx = np.zeros((H, S, DK), np.float32)
        mrow = mask[b, 0, 0, :]
        for h in range(H):
            sc = (q[h] @ k[h].T) / math.sqrt(DK)
            sc = np.where(mrow[None, :] == 0, np.float32(-1e9), sc)
            e = np.exp(sc - sc.max(-1, keepdims=True))
            p = e / e.sum(-1, keepdims=True)
            ctx[h] = p @ v[h]
        cx = ctx.transpose(1, 0, 2).reshape(S, D)
        x1 = ln(xb + cx @ wo + bo, g1, bt1)
        ff = np.maximum(x1 @ w1 + b1, 0.0) @ w2 + b2
        out[b] = ln(x1 + ff, g2, bt2)
    return out


def kernel(**inputs) -> np.ndarray:
    from concourse.bass_utils import run_bass_kernel_spmd

    x = np.asarray(inputs["x"], np.float32)
    mask = np.asarray(inputs["mask"])
    wq, wk, wv, wo = (np.asarray(inputs[k], np.float32)
                      for k in ("wq", "wk", "wv", "wo"))
    w1 = np.asarray(inputs["w1"], np.float32)
    w2 = np.asarray(inputs["w2"], np.float32)
    g1 = float(np.asarray(inputs["g1"]))
    bt1 = float(np.asarray(inputs["bt1"]))
    g2 = float(np.asarray(inputs["g2"]))
    bt2 = float(np.asarray(inputs["bt2"]))
    biases = [np.asarray(inputs[k], np.float32)
              for k in ("bq", "bk", "bv", "bo", "b1", "b2")]

    if any(np.abs(b).max() > 0 for b in biases):
        # biases are identically zero for this module's init; exact but slow
        # host fallback keeps the kernel fully general.
        return _reference_numpy(
            x, mask, wq, biases[0], wk, biases[1], wv, biases[2], wo,
            biases[3], w1, biases[4], w2, biases[5], g1, bt1, g2, bt2)

    mask_has_zeros = bool((mask == 0).any())
    nc = _get_nc(mask_has_zeros)

    wqb, wkb, wvb, wob = (w.astype(NB) for w in (wq, wk, wv, wo))
    w1b = w1.astype(NB)
    w2b = w2.astype(NB)
    cvec = np.array([[g1, -g1 / D, g1 * EPS + bt1,
                      g2, -g2 / D, g2 * EPS + bt2, 0.0, 0.0]], np.float32)

    in_maps = []
    for core in range(N_CORES):
        b = core // 2
        qoff = (core % 2) * M
        xT = np.ascontiguousarray(x[b].T)
        mrow = np.where(mask[b, 0, 0, :] == 0, np.float32(-1e9),
                        np.float32(0.0)).astype(np.float32)
        if qoff:
            # rotate keys so this core's queries sit at columns [0, M)
            xT_k = np.concatenate([xT[:, M:], xT[:, :M]], axis=1)
            mrow = np.concatenate([mrow[M:], mrow[:M]])
        else:
            xT_k = xT
        in_maps.append({
            "xtb": xT_k.astype(NB),
            "wqb": wqb, "wkb": wkb, "wvb": wvb, "wob": wob,
            "w1b": w1b, "w2b": w2b,
            "maskb": np.ascontiguousarray(mrow.reshape(KT, 128).T),
            "cvec": cvec,
        })

    res = run_bass_kernel_spmd(nc, in_maps, core_ids=list(range(N_CORES)))
    globals()["_LAST_RESULTS"] = res

    out = np.empty((B, S, D), np.float32)
    for core in range(N_CORES):
        b = core // 2
        qoff = (core % 2) * M
        out[b, qoff:qoff + M, :] = res.results[core]["outT"].T
    return out


if __name__ == "__main__":
    d = np.load("/root/problem/ref_cache.npz")
    inputs = {k: d[k] for k in d.files if k != "exp"}
    got = kernel(**inputs)
    exp = d["exp"]
    err = np.abs(got - exp)
    print("max abs err:", err.max())
    print("rel max:", err.max() / np.abs(exp).max())
    print("rel l2:", np.linalg.norm(err) / np.linalg.norm(exp))
